# revision 9
# baseline (speedup 1.0000x reference)
"""Trainium2 Bass kernel for nn_GAT_origin (B=32, N=512, 2-layer GAT + Q head).

Sharding: data-parallel over batch across 8 cores (4 batches/core).

Per-core math (batch b):
  WhT[h*8+d, j]  (via PE), f1/f2 rows (via blockdiag matmul),
  attention tiles in transposed layout pT[j, i] = mask^T * exp(leaky(f1_i + f2_j))
  using exp(leaky(x)) = max(exp(x), exp(0.2x)); exp via ACT with
  in = ones x f1 (PE rank-1 outer in PSUM), bias = f2 column, scale in {1, 0.2}.
  Row sums ride the h = p @ Wh matmul as a ones column in the weights.
  Second attention layer identical with 1 "head" (D=32).
  Q head: q = out2_flat @ W_q + b_q with W_q rows reordered host-side so the
  contraction tiles line up with out2 tiles; all compute in bf16 matmuls.
"""
import numpy as np
import ml_dtypes
from contextlib import ExitStack

import concourse.bass as bass
import concourse.tile as tile
from concourse import bacc, mybir
from concourse.bass_utils import run_bass_kernel_spmd

B, N, F_IN = 32, 512, 16
H, DH, DOUT = 8, 8, 32
NCORES = 8
BPC = B // NCORES  # batches per core
P = 128
NJC = N // P  # 4
BF = mybir.dt.bfloat16
F32 = mybir.dt.float32
bf16 = ml_dtypes.bfloat16

Exp = mybir.ActivationFunctionType.Exp
MAX = mybir.AluOpType.max
MIN = mybir.AluOpType.min
ADD = mybir.AluOpType.add
MULT = mybir.AluOpType.mult


def build_kernel(nc):
    ins = {}

    def din(name, shape, dt):
        ins[name] = nc.dram_tensor(name, shape, dt, kind="ExternalInput").ap()

    din("xvt", (F_IN, BPC * N), F32)
    din("adjc", (BPC, N, N), BF)
    din("wcat64", (F_IN, H * DH), F32)
    din("wcat72", (F_IN, H * 9), F32)
    din("ablkaug", (H * DH + 1, 24), F32)
    din("woutaug", (H * DH, DOUT + 1), F32)
    din("ablk2", (DOUT + 1, 3), F32)
    din("ident", (P, P), F32)
    din("identbf", (P, P), BF)
    din("wqr", (DOUT * N, N), BF)
    din("bq", (1, N), BF)
    din("ones4", (1, BPC), BF)
    din("sel8", (H, H * P), BF)
    q_out = nc.dram_tensor("q", (BPC, N), F32, kind="ExternalOutput").ap()

    with tile.TileContext(nc) as tc:
        with ExitStack() as ctx:
            _body(ctx, tc, q_out, ins)
    nc.compile()
    return nc


def _body(ctx, tc, q_out, ins):
    nc = tc.nc
    const = ctx.enter_context(tc.tile_pool(name="const", bufs=1))
    wqp = ctx.enter_context(tc.tile_pool(name="wqp", bufs=128))
    pb = ctx.enter_context(tc.tile_pool(name="pb", bufs=1))
    pjc = ctx.enter_context(tc.tile_pool(name="pjc", bufs=8))
    hot = ctx.enter_context(tc.tile_pool(name="hot", bufs=3))
    o2p = ctx.enter_context(tc.tile_pool(name="o2p", bufs=4))
    small = ctx.enter_context(tc.tile_pool(name="small", bufs=2))

    ps_f1b = ctx.enter_context(tc.tile_pool(name="ps_f1b", bufs=2, space="PSUM"))
    ps_h = ctx.enter_context(tc.tile_pool(name="ps_h", bufs=2, space="PSUM"))
    ps_misc = ctx.enter_context(tc.tile_pool(name="ps_misc", bufs=3, space="PSUM"))
    ps_q = ctx.enter_context(tc.tile_pool(name="ps_q", bufs=1, space="PSUM"))

    # ---- constants ----
    def cload(name, shape, dt):
        t = const.tile(list(shape), dt, tag=name)
        nc.sync.dma_start(t[:], ins[name][:])
        return t

    c_wcat64 = cload("wcat64", (F_IN, H * DH), F32)
    c_wcat72 = cload("wcat72", (F_IN, H * 9), F32)
    c_ablkaug = cload("ablkaug", (H * DH + 1, 24), F32)
    c_woutaug = cload("woutaug", (H * DH, DOUT + 1), F32)
    c_ablk2 = cload("ablk2", (DOUT + 1, 3), F32)
    c_id = cload("ident", (P, P), F32)
    c_idbf = cload("identbf", (P, P), BF)
    c_bq = cload("bq", (1, N), BF)
    c_ones4 = cload("ones4", (1, BPC), BF)
    c_sel8 = cload("sel8", (H, H * P), BF)
    xvt = cload("xvt", (F_IN, BPC * N), F32)
    ones_row = const.tile([1, P], BF, tag="ones_row")
    nc.vector.memset(ones_row[:], 1.0)

    # persistent output tiles [128 n, (b, d)] bf16 per i-chunk
    out2n = []
    for ic in range(NJC):
        t = o2p.tile([P, BPC * DOUT], BF, tag="out2n")
        out2n.append(t)

    # W_q chunk tiles (DMA spread across batches below)
    wq_tiles = [None] * 128

    def load_wq(c):
        t = wqp.tile([P, N], BF, tag="wq")
        nc.sync.dma_start(t[:], ins["wqr"][c * P:(c + 1) * P, :])
        wq_tiles[c] = t

    q_ps = ps_q.tile([BPC, N], F32, tag="q")

    for b in range(BPC):
        # stagger W_q prefetch
        for c in range(32 * b, 32 * (b + 1)):
            load_wq(c)

        xs = xvt[:, b * N:(b + 1) * N]

        # WhT-cat [64, N] (+ ones row 64)
        whtaug = pb.tile([H * DH + 1, N], F32, tag="whtaug")
        mm1 = ps_misc.tile([H * DH, N], F32, tag="mm")
        nc.tensor.matmul(mm1[:], c_wcat64[:], xs, start=True, stop=True)
        nc.vector.tensor_copy(whtaug[0:H * DH, :], mm1[:])
        nc.vector.memset(whtaug[H * DH:H * DH + 1, :], 1.0)

        # f rows [24, N]: 0-7 f1, 8-15 f2, 16-23 0.2*f2
        f_sb = pb.tile([24, N], F32, tag="f_sb")
        mm2 = ps_misc.tile([24, N], F32, tag="mm")
        nc.tensor.matmul(mm2[:], c_ablkaug[:], whtaug[:], start=True, stop=True)
        nc.vector.tensor_copy(f_sb[:], mm2[:])
        f_bf = pb.tile([H, N], BF, tag="f_bf")
        nc.vector.tensor_copy(f_bf[:], f_sb[0:H, :])

        # fcolT per jc: [128, 24] f32
        fcolT = []
        for jc in range(NJC):
            tp = ps_misc.tile([P, 24], F32, tag="mm")
            nc.tensor.transpose(tp[:], f_sb[:, jc * P:(jc + 1) * P], c_id[0:24, 0:24])
            t = pjc.tile([P, 24], F32, tag="fcolT")
            nc.vector.tensor_copy(t[:], tp[:])
            fcolT.append(t)

        # Whb per jc: [128, 72] bf16 with ones at cols 8::9
        whb = []
        for jc in range(NJC):
            mp = ps_misc.tile([P, H * 9], F32, tag="mm")
            nc.tensor.matmul(mp[:], xs[:, jc * P:(jc + 1) * P], c_wcat72[:],
                             start=True, stop=True)
            t = pjc.tile([P, H * 9], BF, tag="whb")
            nc.vector.tensor_copy(t[:], mp[:])
            nc.vector.memset(t[:, 8::9], 1.0)
            whb.append(t)

        # adj natural + transposed (bf16)
        adjnat = []
        for icx in range(NJC):
            t = pjc.tile([P, N], BF, tag="adjnat")
            adj_flat = ins["adjc"].rearrange("b n m -> (b n) m")
            nc.sync.dma_start(t[:], adj_flat[b * N + icx * P:b * N + (icx + 1) * P, :])
            adjnat.append(t)
        adjT = []
        for jc in range(NJC):
            ap_ps = ps_misc.tile([P, N], BF, tag="mm")
            for icx in range(NJC):
                nc.tensor.transpose(ap_ps[:, icx * P:(icx + 1) * P],
                                    adjnat[icx][:, jc * P:(jc + 1) * P], c_idbf[:])
            t = pjc.tile([P, N], BF, tag="adjT")
            nc.vector.tensor_copy(t[:], ap_ps[:])
            adjT.append(t)

        # ---- layer 1 heads ----
        hTall = []
        for ic in range(NJC):
            t = small.tile([P, H * 9], F32, tag=f"hTall{ic}")
            hTall.append(t)
        for h in range(H):
            f1b = ps_f1b.tile([P, N], F32, tag="f1b")
            nc.tensor.matmul(f1b[:], c_sel8[:, h * P:(h + 1) * P], f_bf[:],
                             start=True, stop=True)
            h_ps = ps_h.tile([9, N], F32, tag="h")
            for jc in range(NJC):
                A = hot.tile([P, N], BF, tag="A")
                nc.scalar.activation(A[:], f1b[:], Exp,
                                     bias=fcolT[jc][:, 8 + h:9 + h], scale=1.0)
                Bt = hot.tile([P, N], BF, tag="B")
                nc.scalar.activation(Bt[:], f1b[:], Exp,
                                     bias=fcolT[jc][:, 16 + h:17 + h], scale=0.2)
                C = hot.tile([P, N], BF, tag="C")
                nc.vector.tensor_tensor(out=C[:], in0=A[:], in1=Bt[:], op=MAX)
                p = hot.tile([P, N], BF, tag="p")
                nc.gpsimd.tensor_tensor(out=p[:], in0=C[:], in1=adjT[jc][:], op=MULT)
                nc.tensor.matmul(h_ps[:], whb[jc][:, 9 * h:9 * h + 9], p[:],
                                 start=(jc == 0), stop=(jc == NJC - 1))
            hsb = small.tile([9, N], F32, tag="hsb")
            nc.vector.tensor_copy(hsb[:], h_ps[:])
            for ic in range(NJC):
                tp = ps_misc.tile([P, 9], F32, tag="mm")
                nc.tensor.transpose(tp[:], hsb[:, ic * P:(ic + 1) * P],
                                    c_id[0:9, 0:9])
                nc.vector.tensor_copy(hTall[ic][:, 9 * h:9 * h + 9], tp[:])

        # normalize + elu -> hcatN [128, 64] f32 per ic; build hcatNT [64, N]
        hcatNT = pb.tile([H * DH, N], F32, tag="hcatNT")
        for ic in range(NJC):
            hT = hTall[ic]
            rec8 = small.tile([P, H], F32, tag="rec8")
            nc.vector.reciprocal(rec8[:], hT[:, 8::9])
            hn = small.tile([P, H * DH], F32, tag="hn")
            hT3 = hT[:].rearrange("p (h x) -> p h x", x=9)[:, :, 0:DH]
            rec3 = rec8[:].unsqueeze(2).broadcast_to([P, H, DH])
            hn3 = hn[:].rearrange("p (h d) -> p h d", d=DH)
            nc.vector.tensor_tensor(out=hn3, in0=hT3, in1=rec3, op=MULT)
            # elu
            E = small.tile([P, H * DH], F32, tag="E")
            nc.scalar.activation(E[:], hn[:], Exp)
            Em = small.tile([P, H * DH], F32, tag="Em")
            nc.vector.tensor_scalar(out=Em[:], in0=E[:], scalar1=-1.0, scalar2=0.0,
                                    op0=ADD, op1=MIN)
            hcatN = small.tile([P, H * DH], F32, tag="hcatN")
            nc.vector.scalar_tensor_tensor(out=hcatN[:], in0=hn[:], scalar=0.0,
                                           in1=Em[:], op0=MAX, op1=ADD)
            tp2 = ps_misc.tile([H * DH, P], F32, tag="mm")
            nc.tensor.transpose(tp2[:], hcatN[:], c_id[:])
            nc.vector.tensor_copy(hcatNT[:, ic * P:(ic + 1) * P], tp2[:])

        # ---- layer 2 ----
        whotaug = pb.tile([DOUT + 1, N], F32, tag="whotaug")
        mm3 = ps_misc.tile([DOUT, N], F32, tag="mm")
        nc.tensor.matmul(mm3[:], c_woutaug[:, 0:DOUT], hcatNT[:], start=True, stop=True)
        nc.vector.tensor_copy(whotaug[0:DOUT, :], mm3[:])
        nc.vector.memset(whotaug[DOUT:DOUT + 1, :], 1.0)

        whob = []
        for jc in range(NJC):
            mp = ps_misc.tile([P, DOUT + 1], F32, tag="mm")
            nc.tensor.matmul(mp[:], hcatNT[:, jc * P:(jc + 1) * P], c_woutaug[:],
                             start=True, stop=True)
            t = pjc.tile([P, DOUT + 1], BF, tag="whob")
            nc.vector.tensor_copy(t[:], mp[:])
            nc.vector.memset(t[:, DOUT:DOUT + 1], 1.0)
            whob.append(t)

        g_sb = pb.tile([3, N], F32, tag="g_sb")
        mm4 = ps_misc.tile([3, N], F32, tag="mm")
        nc.tensor.matmul(mm4[:], c_ablk2[:], whotaug[:], start=True, stop=True)
        nc.vector.tensor_copy(g_sb[:], mm4[:])
        g_bf = pb.tile([1, N], BF, tag="g_bf")
        nc.vector.tensor_copy(g_bf[:], g_sb[0:1, :])
        gcolT = []
        for jc in range(NJC):
            tp = ps_misc.tile([P, 3], F32, tag="mm")
            nc.tensor.transpose(tp[:], g_sb[:, jc * P:(jc + 1) * P], c_id[0:3, 0:3])
            t = pjc.tile([P, 3], F32, tag="gcolT")
            nc.vector.tensor_copy(t[:], tp[:])
            gcolT.append(t)

        g1b = ps_f1b.tile([P, N], F32, tag="f1b")
        nc.tensor.matmul(g1b[:], ones_row[:], g_bf[:], start=True, stop=True)
        o_ps = ps_h.tile([DOUT + 1, N], F32, tag="h")
        for jc in range(NJC):
            A = hot.tile([P, N], BF, tag="A")
            nc.scalar.activation(A[:], g1b[:], Exp, bias=gcolT[jc][:, 1:2], scale=1.0)
            Bt = hot.tile([P, N], BF, tag="B")
            nc.scalar.activation(Bt[:], g1b[:], Exp, bias=gcolT[jc][:, 2:3], scale=0.2)
            C = hot.tile([P, N], BF, tag="C")
            nc.vector.tensor_tensor(out=C[:], in0=A[:], in1=Bt[:], op=MAX)
            p = hot.tile([P, N], BF, tag="p")
            nc.gpsimd.tensor_tensor(out=p[:], in0=C[:], in1=adjT[jc][:], op=MULT)
            nc.tensor.matmul(o_ps[:], whob[jc][:], p[:],
                             start=(jc == 0), stop=(jc == NJC - 1))
        o_sb = pb.tile([DOUT + 1, N], F32, tag="o_sb")
        nc.vector.tensor_copy(o_sb[:], o_ps[:])

        for ic in range(NJC):
            tp = ps_misc.tile([P, DOUT + 1], F32, tag="mm")
            nc.tensor.transpose(tp[:], o_sb[:, ic * P:(ic + 1) * P],
                                c_id[0:DOUT + 1, 0:DOUT + 1])
            oT = small.tile([P, DOUT + 1], F32, tag="oT")
            nc.vector.tensor_copy(oT[:], tp[:])
            rec1 = small.tile([P, 1], F32, tag="rec1")
            nc.vector.reciprocal(rec1[:], oT[:, DOUT:DOUT + 1])
            on = small.tile([P, DOUT], F32, tag="on")
            nc.vector.tensor_scalar(out=on[:], in0=oT[:, 0:DOUT], scalar1=rec1[:],
                                    scalar2=None, op0=MULT)
            E = small.tile([P, DOUT], F32, tag="E2")
            nc.scalar.activation(E[:], on[:], Exp)
            Em = small.tile([P, DOUT], F32, tag="Em2")
            nc.vector.tensor_scalar(out=Em[:], in0=E[:], scalar1=-1.0, scalar2=0.0,
                                    op0=ADD, op1=MIN)
            nc.vector.scalar_tensor_tensor(
                out=out2n[ic][:, b * DOUT:(b + 1) * DOUT], in0=on[:], scalar=0.0,
                in1=Em[:], op0=MAX, op1=ADD)

    # ---- Q head ----
    nc.tensor.matmul(q_ps[:], c_ones4[:], c_bq[:], start=True, stop=False)
    for c in range(128):
        d, ic = divmod(c, NJC)
        lhsT = out2n[ic][:, d::DOUT]
        nc.tensor.matmul(q_ps[:], lhsT, wq_tiles[c][:],
                         start=False, stop=(c == 127))
    q_sb = const.tile([BPC, N], F32, tag="q_sb")
    nc.vector.tensor_copy(q_sb[:], q_ps[:])
    nc.sync.dma_start(q_out[:], q_sb[:])


_CACHE = {}


def _get_nc():
    if "nc" not in _CACHE:
        nc = bacc.Bacc("TRN2", target_bir_lowering=False, debug=False)
        _CACHE["nc"] = build_kernel(nc)
    return _CACHE["nc"]


def prep_inputs(xv, adj, W_heads, a1, a2, W_out, a1_out, a2_out, W_q, b_q):
    """Host-side prep: small-parameter rearrangement + per-core sharding."""
    wcat64 = np.ascontiguousarray(
        W_heads.transpose(1, 0, 2).reshape(F_IN, H * DH)).astype(np.float32)
    wcat72 = np.zeros((F_IN, H * 9), np.float32)
    for h in range(H):
        wcat72[:, 9 * h:9 * h + DH] = W_heads[h]
    ablkaug = np.zeros((H * DH + 1, 24), np.float32)
    for h in range(H):
        ablkaug[h * DH:(h + 1) * DH, h] = a1[h]
        ablkaug[h * DH:(h + 1) * DH, 8 + h] = a2[h]
        ablkaug[h * DH:(h + 1) * DH, 16 + h] = 0.2 * a2[h]
    woutaug = np.zeros((H * DH, DOUT + 1), np.float32)
    woutaug[:, 0:DOUT] = W_out
    ablk2 = np.zeros((DOUT + 1, 3), np.float32)
    ablk2[0:DOUT, 0] = a1_out
    ablk2[0:DOUT, 1] = a2_out
    ablk2[0:DOUT, 2] = 0.2 * a2_out
    ident = np.eye(P, dtype=np.float32)
    identbf = np.eye(P, dtype=bf16)
    wqr = np.ascontiguousarray(
        W_q.reshape(N, DOUT, N).transpose(1, 0, 2).reshape(DOUT * N, N)).astype(bf16)
    bq = b_q.reshape(1, N).astype(bf16)
    ones4 = np.ones((1, BPC), bf16)
    sel8 = np.zeros((H, H * P), bf16)
    for h in range(H):
        sel8[h, h * P:(h + 1) * P] = 1

    shared = dict(wcat64=wcat64, wcat72=wcat72, ablkaug=ablkaug, woutaug=woutaug,
                  ablk2=ablk2, ident=ident, identbf=identbf, wqr=wqr, bq=bq,
                  ones4=ones4, sel8=sel8)
    in_maps = []
    for c in range(NCORES):
        bs = slice(c * BPC, (c + 1) * BPC)
        xvt = np.ascontiguousarray(
            xv[bs].transpose(2, 0, 1).reshape(F_IN, BPC * N)).astype(np.float32)
        adjc = adj[bs].astype(bf16)
        in_maps.append(dict(shared, xvt=xvt, adjc=adjc))
    return in_maps


def kernel(xv, adj, W_heads, a1, a2, W_out, a1_out, a2_out, W_q, b_q,
           trace=False):
    nc = _get_nc()
    in_maps = prep_inputs(xv, adj, W_heads, a1, a2, W_out, a1_out, a2_out,
                          W_q, b_q)
    res = run_bass_kernel_spmd(nc, in_maps, core_ids=list(range(NCORES)),
                               trace=trace)
    _CACHE["last_result"] = res
    q = np.concatenate([res.results[c]["q"] for c in range(NCORES)], axis=0)
    return q.reshape(B, N, 1).astype(np.float32)


def build_noop_nc():
    """Tiny kernel (copy [4,512]) for dispatch-overhead baseline."""
    nc = bacc.Bacc("TRN2", target_bir_lowering=False, debug=False)
    x = nc.dram_tensor("x", (BPC, N), F32, kind="ExternalInput").ap()
    y = nc.dram_tensor("y", (BPC, N), F32, kind="ExternalOutput").ap()
    with tile.TileContext(nc) as tc:
        with ExitStack() as ctx:
            pool = ctx.enter_context(tc.tile_pool(name="p", bufs=1))
            t = pool.tile([BPC, N], F32, tag="t")
            nc.sync.dma_start(t[:], x[:])
            nc.sync.dma_start(y[:], t[:])
    nc.compile()
    return nc


def make_timed_runner(nc, in_maps):
    """Build a repeat-callable PJRT runner with device-resident inputs.

    Mirrors bass2jax.run_bass_via_pjrt's multi-core branch but without
    donation so the same buffers can be re-executed for timing.
    """
    import jax
    from jax.sharding import Mesh, NamedSharding, PartitionSpec
    from jax.experimental.shard_map import shard_map
    import concourse.mybir as mybir_
    from concourse.bass2jax import (_bass_exec_p, install_neuronx_cc_hook,
                                    partition_id_tensor)

    install_neuronx_cc_hook()
    n_cores = len(in_maps)
    partition_name = (nc.partition_id_tensor.name
                      if nc.partition_id_tensor else None)
    in_names, out_names, out_avals, zero_outs = [], [], [], []
    for alloc in nc.m.functions[0].allocations:
        if not isinstance(alloc, mybir_.MemoryLocationSet):
            continue
        name = alloc.memorylocations[0].name
        if alloc.kind == "ExternalInput":
            if name != partition_name:
                in_names.append(name)
        elif alloc.kind == "ExternalOutput":
            out_names.append(name)
            shape = tuple(alloc.tensor_shape)
            dtype = mybir_.dt.np(alloc.dtype)
            out_avals.append(jax.core.ShapedArray(shape, dtype))
            zero_outs.append(np.zeros(shape, dtype))
    n_params = len(in_names)
    all_names = in_names + out_names
    if partition_name is not None:
        all_names = all_names + [partition_name]

    def _bd(*args):
        operands = list(args)
        if partition_name is not None:
            operands.append(partition_id_tensor())
        outs = _bass_exec_p.bind(
            *operands,
            out_avals=tuple(out_avals),
            in_names=tuple(all_names),
            out_names=tuple(out_names),
            lowering_input_output_aliases=(),
            sim_require_finite=True,
            sim_require_nnan=True,
            nc=nc,
        )
        return tuple(outs)

    devices = jax.devices()[:n_cores]
    mesh = Mesh(np.asarray(devices), ("core",))
    spec = PartitionSpec("core")
    nshard = NamedSharding(mesh, spec)
    sharded = jax.jit(
        shard_map(_bd, mesh=mesh,
                  in_specs=(spec,) * (n_params + len(out_names)),
                  out_specs=(spec,) * len(out_names),
                  check_rep=False),
        keep_unused=True,
    )
    concat_in = [
        jax.device_put(
            np.concatenate([np.asarray(in_maps[c][nm]) for c in range(n_cores)], 0),
            nshard)
        for nm in in_names
    ]
    concat_zeros = [
        jax.device_put(np.zeros((n_cores * z.shape[0], *z.shape[1:]), z.dtype), nshard)
        for z in zero_outs
    ]

    def run():
        outs = sharded(*concat_in, *concat_zeros)
        jax.block_until_ready(outs)
        return outs

    return run
